# revision 1
# baseline (speedup 1.0000x reference)
"""Trainium2 Bass kernel for GraphTransformerLinkPredictor — v3.

Sharding: edges sharded by target-node range (core c owns targets
[c*NLOC, (c+1)*NLOC)); node linears data-parallel over the same ranges;
k/v quantized to fp8 and AllGathered each layer; segment softmax +
aggregation local per core.

v1 measured 8.30 ms, GpSimd 65% busy on INDIRECT1D: every per-subtile
indirect gather costs ~1.05us of serialized SWDGE descriptor-generation
time. Multi-offset indirect DMAs and all extended gather instructions
(dma_gather & co.) are unavailable on this image, so the per-subtile
kv gather is the hard floor. v3 removes everything else:
  - q is never gathered: q stays resident in SBUF ([128, NT*128] slab
    written by the node phase); per edge subtile, q[col] is selected by
    the PE as ohT^T @ Q_tile using a CPU-precomputed transposed one-hot
    (col is tile-local by construction). PSUM->SBUF copies batched 4
    subtiles per PSUM bank. This halves the INDIRECT1D count.
  - kv table fp8(e4m3) in HBM: halves AllGather bytes (the gather
    casts fp8->bf16 in the DMA, so DVE stays in 2x mode).
  - exp() broadcast per-head on ACT (stride-0 input AP) so the v*ex
    multiply runs in DVE 2x mode.
  - x/rw pre-transposed on CPU; h transposed via DMA-transpose loads;
    no PE transposes in the node phase.
  - LayerNorm rsqrt batched per layer (one ACT Sqrt on [128, NT]); ReLU
    folded into a DVE tensor_scalar (mult, max) op. No activation-table
    thrashing.
  - Node-phase DMAs batched into GT-tile slabs; DVE/ACT edge ops
    batched KB subtiles per instruction.
"""

import math
import os
from contextlib import ExitStack

import numpy as np

P = 128
HID = 128
HEADS = 4
DH = 32
L = 2
EPS = 1e-5
NCORES = 8
KB = 8    # edge subtiles per DVE/ACT batch
QS = 4    # qsel matmuls per PSUM bank (one ACT copy per QS subtiles)
GT = 8    # node tiles per linear-phase group
KBP = 8   # pair subtiles per DVE batch


def _groups(nt, g):
    out = []
    t0 = 0
    while t0 < nt:
        out.append((t0, min(g, nt - t0)))
        t0 += min(g, nt - t0)
    return out


def _prep(inputs):
    import ml_dtypes

    bf = ml_dtypes.bfloat16
    f8 = ml_dtypes.float8_e4m3

    x = np.ascontiguousarray(np.asarray(inputs["x"], dtype=np.float32))
    rw = np.ascontiguousarray(np.asarray(inputs["rw_diag"], dtype=np.float32))
    ei = np.asarray(inputs["edge_index"]).astype(np.int64)
    psrc = np.asarray(inputs["src"]).astype(np.int64)
    pdst = np.asarray(inputs["dst"]).astype(np.int64)

    N = x.shape[0]
    IN_C = x.shape[1]
    RWD = rw.shape[1]
    NT = math.ceil(N / (NCORES * P))
    NLOC = NT * P
    NPADT = NLOC * NCORES

    W_rwse = np.asarray(inputs["W_rwse"], np.float32)
    b_rwse = np.asarray(inputs["b_rwse"], np.float32)
    W_in = np.asarray(inputs["W_in"], np.float32)
    b_in = np.asarray(inputs["b_in"], np.float32)
    W1 = np.ascontiguousarray(W_in[:IN_C]).astype(bf)
    W2 = np.ascontiguousarray(W_rwse @ W_in[IN_C:]).astype(bf)
    bcat = (b_in + b_rwse @ W_in[IN_C:]).astype(np.float32)

    Wq = np.asarray(inputs["Wq"], np.float32)
    Wk = np.asarray(inputs["Wk"], np.float32)
    Wv = np.asarray(inputs["Wv"], np.float32)
    Ws = np.asarray(inputs["Ws"], np.float32)
    bq = np.asarray(inputs["bq"], np.float32)
    bk = np.asarray(inputs["bk"], np.float32)
    bv = np.asarray(inputs["bv"], np.float32)
    bs = np.asarray(inputs["bs"], np.float32)
    ln_g = np.asarray(inputs["ln_g"], np.float32)
    ln_b = np.asarray(inputs["ln_b"], np.float32)

    Wcat = [np.ascontiguousarray(np.concatenate(
        [Wq[l], Wk[l], Wv[l], Ws[l]], axis=1)).astype(bf) for l in range(L)]
    bqkvs = [np.concatenate([bq[l], bk[l], bv[l], bs[l]]) for l in range(L)]

    row = ei[0]
    col = ei[1]
    core_of = col // NLOC
    tile_of = (col % NLOC) // P

    order = np.lexsort((row, tile_of, core_of))
    srow = row[order]
    scol = col[order]

    flat = core_of[order] * NT + tile_of[order]
    cnt = np.bincount(flat, minlength=NCORES * NT).reshape(NCORES, NT)
    # per-core local-tile permutation: slot s handles each core's s-th
    # biggest tile, so the shared (max-over-cores) subtile counts align
    # big-with-big and the padding shrinks. All device addressing is
    # slot-based; the permutation lives purely in the CPU-side data
    # (x/rw tile order, gather index values).
    perm = np.argsort(-cnt, axis=1, kind="stable")      # [NC, NT] slot->tile
    inv_perm = np.argsort(perm, axis=1)                 # [NC, NT] tile->slot
    cnt_s = np.take_along_axis(cnt, perm, axis=1)       # counts per slot
    tcnt = np.maximum(1, np.ceil(cnt_s / P).astype(np.int64)).max(axis=0)
    total_t = int(tcnt.sum())
    tcnt[-1] += (-total_t) % KB
    ET = int(tcnt.sum())
    NB = ET // KB

    # map a global node id to its row in the slot-ordered tables
    def slot_row(g):
        gc = g // NLOC
        gl = g % NLOC
        return gc * NLOC + inv_perm[gc, gl // P] * P + gl % P

    sub_tile = np.repeat(np.arange(NT), tcnt)
    starts = np.concatenate([[0], np.cumsum(tcnt)])[:-1] * P
    sub_start = np.zeros(ET, bool)
    sub_stop = np.zeros(ET, bool)
    sub_start[starts // P] = True
    sub_stop[(starts // P + tcnt - 1)] = True

    msrow = slot_row(srow)  # kv-table rows are slot-ordered
    kvidx = np.zeros((NCORES, ET * P), np.int32)
    ohm = np.zeros((NCORES, ET * P, P), bf)
    ohtm = np.zeros((NCORES, ET, P, P), bf)  # [sub, node, edge]
    gstart = np.concatenate([[0], np.cumsum(cnt.reshape(-1))])
    for c in range(NCORES):
        for s in range(NT):
            t = perm[c, s]
            g0 = gstart[c * NT + t]
            n = cnt[c, t]
            if n == 0:
                continue
            o = starts[s]
            sl = np.arange(o, o + n)
            kvidx[c, sl] = msrow[g0:g0 + n]
            cl = (scol[g0:g0 + n] - c * NLOC).astype(np.int32)
            ohm[c, sl, cl % P] = 1
            ohtm[c, sl // P, cl % P, sl % P] = 1

    kvidx_hw = kvidx.reshape(NCORES, NB, KB, P).transpose(0, 1, 3, 2)
    kvidx_hw = np.ascontiguousarray(kvidx_hw)
    oh_hw = np.ascontiguousarray(
        ohm.reshape(NCORES, NB, KB, P, P).transpose(0, 1, 3, 2, 4)).reshape(
        NCORES, NB, P, KB * P)
    oht_hw = np.ascontiguousarray(
        ohtm.reshape(NCORES, NB, KB, P, P).transpose(0, 1, 3, 2, 4)).reshape(
        NCORES, NB, P, KB * P)

    # node features (zero-padded, transposed tiles)
    xs = np.zeros((NCORES, NLOC, IN_C), np.float32)
    rws = np.zeros((NCORES, NLOC, RWD), np.float32)
    for c in range(NCORES):
        lo = c * NLOC
        hi = min(N, lo + NLOC)
        if hi > lo:
            xs[c, :hi - lo] = x[lo:hi]
            rws[c, :hi - lo] = rw[lo:hi]
    ci = np.arange(NCORES)[:, None]
    xT_hw = np.ascontiguousarray(
        xs.reshape(NCORES, NT, P, IN_C)[ci, perm].transpose(
            0, 1, 3, 2)).astype(bf)
    rwT_hw = np.ascontiguousarray(
        rws.reshape(NCORES, NT, P, RWD)[ci, perm].transpose(
            0, 1, 3, 2)).astype(bf)

    # pairs: contiguous shards per core
    NPAIR = psrc.shape[0]
    PLOC = math.ceil(NPAIR / NCORES)
    plocs = [max(0, min(PLOC, NPAIR - c * PLOC)) for c in range(NCORES)]
    NPS = math.ceil(PLOC / P)
    NPB = math.ceil(NPS / KBP)
    PPAD = NPB * KBP * P
    ps = np.zeros((NCORES, PPAD), np.int32)
    pd = np.zeros((NCORES, PPAD), np.int32)
    mpsrc = slot_row(psrc)
    mpdst = slot_row(pdst)
    for c in range(NCORES):
        n = plocs[c]
        ps[c, :n] = mpsrc[c * PLOC:c * PLOC + n]
        pd[c, :n] = mpdst[c * PLOC:c * PLOC + n]
    pidx_hw = np.zeros((NCORES, NPB, P, 2 * KBP), np.int32)
    pidx_hw[:, :, :, 0:KBP] = ps.reshape(
        NCORES, NPB, KBP, P).transpose(0, 1, 3, 2)
    pidx_hw[:, :, :, KBP:] = pd.reshape(
        NCORES, NPB, KBP, P).transpose(0, 1, 3, 2)

    return dict(
        N=N, IN_C=IN_C, RWD=RWD, NT=NT, NLOC=NLOC, NPADT=NPADT,
        ET=ET, NB=NB, NPB=NPB, PLOC=PLOC, plocs=plocs, NPAIR=NPAIR,
        W1=W1, W2=W2, bcat=bcat, Wcat=Wcat, bqkvs=bqkvs,
        ln_g=ln_g, ln_b=ln_b,
        xT=xT_hw, rwT=rwT_hw, kvidx=kvidx_hw, oh=oh_hw, oht=oht_hw,
        pidx=pidx_hw,
        sub_tile=sub_tile, sub_start=sub_start, sub_stop=sub_stop,
    )


def _build(pr):
    import concourse.bass as bass
    import concourse.bacc as bacc
    import concourse.mybir as mybir
    import concourse.tile as tile

    f32 = mybir.dt.float32
    bf16 = mybir.dt.bfloat16
    fp8 = mybir.dt.float8e4
    i32 = mybir.dt.int32
    ALU = mybir.AluOpType
    ACT = mybir.ActivationFunctionType
    X = mybir.AxisListType.X

    NT, NLOC, NPADT = pr["NT"], pr["NLOC"], pr["NPADT"]
    NB, NPB = pr["NB"], pr["NPB"]
    IN_C, RWD = pr["IN_C"], pr["RWD"]
    scale = 1.0 / math.sqrt(DH)

    bias_nz = bool(np.any(pr["bcat"] != 0))
    qkvs_nz = [bool(np.any(b != 0)) for b in pr["bqkvs"]]
    g_one = [bool(np.all(pr["ln_g"][l] == 1)) for l in range(L)]
    b_zero = [bool(np.all(pr["ln_b"][l] == 0)) for l in range(L)]

    nc = bacc.Bacc(None, num_devices=NCORES)

    t_xT = nc.dram_tensor("x_t", [NT, IN_C, P], bf16, kind="ExternalInput")
    t_rwT = nc.dram_tensor("rw_t", [NT, RWD, P], bf16, kind="ExternalInput")
    t_w1 = nc.dram_tensor("w1", [IN_C, HID], bf16, kind="ExternalInput")
    t_w2 = nc.dram_tensor("w2", [RWD, HID], bf16, kind="ExternalInput")
    t_wc = [nc.dram_tensor(f"wc{l}", [HID, 4 * HID], bf16,
                           kind="ExternalInput") for l in range(L)]
    t_kvi = nc.dram_tensor("kvidx", [NB, P, KB], i32, kind="ExternalInput")
    t_oh = nc.dram_tensor("onehot", [NB, P, KB * P], bf16,
                          kind="ExternalInput")
    t_oht = nc.dram_tensor("onehot_t", [NB, P, KB * P], bf16,
                           kind="ExternalInput")
    t_pidx = nc.dram_tensor("pidx", [NPB, P, 2 * KBP], i32,
                            kind="ExternalInput")

    t_h = nc.dram_tensor("h_loc", [NT, P, HID], bf16, kind="Internal")
    t_kvin = nc.dram_tensor("kv_in", [NLOC, 2 * HID], fp8, kind="Internal")
    t_kv = nc.dram_tensor("kv_full", [NPADT, 2 * HID], fp8,
                          kind="Internal", addr_space="Shared")
    t_hfin = nc.dram_tensor("hf_in", [NLOC, HID], fp8, kind="Internal")
    t_hf = nc.dram_tensor("hf_full", [NPADT, HID], fp8, kind="Internal",
                          addr_space="Shared")
    t_out = nc.dram_tensor("out", [NPB * KBP, P], f32,
                           kind="ExternalOutput")

    rg = [list(range(NCORES))]
    groups = _groups(NT, GT)

    with ExitStack() as ctx:
        tc = ctx.enter_context(tile.TileContext(nc))
        cpool = ctx.enter_context(tc.tile_pool(name="const", bufs=1))
        lp = ctx.enter_context(tc.tile_pool(name="lp", bufs=2))
        ep = ctx.enter_context(tc.tile_pool(name="ep", bufs=4))
        epi = ctx.enter_context(tc.tile_pool(name="epi", bufs=2))
        pp_mm = ctx.enter_context(tc.tile_pool(name="ppm", bufs=3,
                                               space="PSUM"))
        pp_agg = ctx.enter_context(tc.tile_pool(name="ppa", bufs=3,
                                                space="PSUM"))
        pp_qs = ctx.enter_context(tc.tile_pool(name="ppq", bufs=2,
                                               space="PSUM"))

        w1 = cpool.tile([IN_C, HID], bf16)
        nc.sync.dma_start(out=w1[:], in_=t_w1[:, :])
        w2 = cpool.tile([RWD, HID], bf16)
        nc.sync.dma_start(out=w2[:], in_=t_w2[:, :])
        wc = []
        for l in range(L):
            w = cpool.tile([HID, 4 * HID], bf16, name=f"wc{l}")
            nc.sync.dma_start(out=w[:], in_=t_wc[l][:, :])
            wc.append(w)

        sh_all = cpool.tile([P, NT * HID], bf16, name="sh_all")
        yc_all = cpool.tile([P, NT * HID], bf16, name="yc_all")
        q_all = cpool.tile([P, NT * HID], bf16, name="q_all")
        var_all = cpool.tile([P, NT], f32, name="var_all")
        isd_all = cpool.tile([P, NT], f32, name="isd_all")

        def linear_phase(l):
            for (g0, gsz) in groups:
                hT_sl = lp.tile([P, GT * P], bf16, tag="hT")
                h_sl = lp.tile([P, GT * P], bf16, tag="h")
                if l == 0:
                    xT_sl = lp.tile([P, GT * P], bf16, tag="xT")
                    nc.sync.dma_start(
                        out=xT_sl[:, :gsz * P],
                        in_=t_xT[g0:g0 + gsz].rearrange("g c n -> c g n"))
                    rwT_sl = lp.tile([RWD, GT * P], bf16, tag="rwT")
                    nc.sync.dma_start(
                        out=rwT_sl[:, :gsz * P],
                        in_=t_rwT[g0:g0 + gsz].rearrange("g c n -> c g n"))
                    for j in range(gsz):
                        sl = slice(j * P, (j + 1) * P)
                        hT_ps = pp_mm.tile([P, P], f32, tag="mm0")
                        nc.tensor.matmul(out=hT_ps[:], lhsT=w1[:],
                                         rhs=xT_sl[:, sl], start=True,
                                         stop=False)
                        nc.tensor.matmul(out=hT_ps[:], lhsT=w2[:],
                                         rhs=rwT_sl[:, sl], start=False,
                                         stop=True)
                        h_ps = pp_mm.tile([P, P], f32, tag="mm0")
                        nc.tensor.matmul(out=h_ps[:], lhsT=xT_sl[:, sl],
                                         rhs=w1[:], start=True, stop=False)
                        nc.tensor.matmul(out=h_ps[:], lhsT=rwT_sl[:, sl],
                                         rhs=w2[:], start=False, stop=True)
                        nc.scalar.copy(out=hT_sl[:, sl], in_=hT_ps[:])
                        nc.vector.tensor_copy(h_sl[:, sl], h_ps[:])
                    if bias_nz:
                        bc_r = cpool.tile([1, HID], f32, name="bc_r")
                        bc_c = cpool.tile([P, 1], f32, name="bc_c")
                        nc.vector.tensor_tensor(
                            out=h_sl[:, :gsz * P].rearrange(
                                "p (g f) -> p g f", f=HID),
                            in0=h_sl[:, :gsz * P].rearrange(
                                "p (g f) -> p g f", f=HID),
                            in1=bc_r[:].to_broadcast([P, gsz, HID]),
                            op=ALU.add)
                        nc.vector.tensor_scalar_add(
                            hT_sl[:, :gsz * P], hT_sl[:, :gsz * P], bc_c[:])
                else:
                    nc.sync.dma_start(
                        out=hT_sl[:, :gsz * P],
                        in_=t_h[g0:g0 + gsz].rearrange("g p f -> (g p) f"),
                        transpose=True)
                    nc.scalar.dma_start(
                        out=h_sl[:, :gsz * P],
                        in_=t_h[g0:g0 + gsz].rearrange("g p f -> p g f"))
                kvs_sl = lp.tile([P, GT * 2 * HID], fp8, tag="kvs")
                for j in range(gsz):
                    t = g0 + j
                    sl = slice(j * P, (j + 1) * P)
                    qk_ps = pp_mm.tile([P, 4 * HID], f32, tag="mm0")
                    nc.tensor.matmul(out=qk_ps[:], lhsT=hT_sl[:, sl],
                                     rhs=wc[l][:], start=True, stop=True)
                    if qkvs_nz[l]:
                        bqk = cpool.tile([1, 4 * HID], f32, name=f"bqk{l}")
                        nc.vector.tensor_tensor(
                            out=qk_ps[:], in0=qk_ps[:],
                            in1=bqk[:].to_broadcast([P, 4 * HID]), op=ALU.add)
                    nc.scalar.copy(out=q_all[:, t * HID:(t + 1) * HID],
                                   in_=qk_ps[:, 0:HID])
                    nc.scalar.copy(
                        out=kvs_sl[:, j * 2 * HID:(j + 1) * 2 * HID],
                        in_=qk_ps[:, HID:3 * HID])
                    nc.vector.tensor_tensor(
                        out=sh_all[:, t * HID:(t + 1) * HID],
                        in0=qk_ps[:, 3 * HID:4 * HID], in1=h_sl[:, sl],
                        op=ALU.add)
                nc.sync.dma_start(
                    out=t_kvin[g0 * P:(g0 + gsz) * P, :].rearrange(
                        "(g p) f -> p g f", p=P),
                    in_=kvs_sl[:, :gsz * 2 * HID])

        def pass1(l, t, agg):
            zsb = epi.tile([P, HEADS], f32, tag="zsb")
            nc.vector.tensor_scalar_max(zsb[:], agg[:, HID:HID + HEADS],
                                        1e-30)
            inv = epi.tile([P, HEADS], f32, tag="inv")
            nc.vector.reciprocal(inv[:], zsb[:])
            y = epi.tile([P, HID], bf16, tag="y")
            nc.vector.tensor_tensor(
                out=y[:].rearrange("p (h d) -> p h d", d=DH),
                in0=agg[:, 0:HID].rearrange("p (h d) -> p h d", d=DH),
                in1=inv[:].unsqueeze(-1).to_broadcast([P, HEADS, DH]),
                op=ALU.mult)
            nc.vector.tensor_tensor(
                out=y[:], in0=y[:], in1=sh_all[:, t * HID:(t + 1) * HID],
                op=ALU.add)
            musum = epi.tile([P, 1], f32, tag="musum")
            nc.vector.reduce_sum(musum[:], y[:], axis=X)
            mu = epi.tile([P, 1], f32, tag="mu")
            nc.vector.tensor_scalar_mul(mu[:], musum[:], 1.0 / HID)
            yc = yc_all[:, t * HID:(t + 1) * HID]
            nc.vector.tensor_scalar_sub(yc, y[:], mu[:])
            junk = epi.tile([P, HID], bf16, tag="junk")
            ssq = epi.tile([P, 1], f32, tag="ssq")
            nc.vector.scalar_tensor_tensor(
                out=junk[:], in0=yc, scalar=1.0, in1=yc,
                op0=ALU.bypass, op1=ALU.mult, accum_out=ssq[:])
            nc.vector.tensor_scalar(
                out=var_all[:, t:t + 1], in0=ssq[:], scalar1=1.0 / HID,
                scalar2=EPS, op0=ALU.mult, op1=ALU.add)

        def finish_group(l, g0, gsz, gen, lng, lnb):
            sdg = epi.tile([P, GT], f32, tag="sdg")
            nc.scalar.activation(out=sdg[:, :gsz],
                                 in_=var_all[:, g0:g0 + gsz], func=ACT.Sqrt)
            nc.vector.reciprocal(isd_all[:, g0:g0 + gsz], sdg[:, :gsz])
            h_sl = lp.tile([P, GT * HID], bf16, tag="ho")
            for j in range(gsz):
                t = g0 + j
                sl = slice(j * HID, (j + 1) * HID)
                ycb = yc_all[:, t * HID:(t + 1) * HID]
                if gen:
                    tmp = epi.tile([P, HID], bf16, tag="tmp")
                    nc.vector.tensor_scalar_mul(
                        tmp[:], ycb, isd_all[:, t:t + 1])
                    nc.vector.tensor_tensor(
                        out=tmp[:], in0=tmp[:],
                        in1=lng[:].to_broadcast([P, HID]), op=ALU.mult)
                    nc.vector.tensor_tensor(
                        out=tmp[:], in0=tmp[:],
                        in1=lnb[:].to_broadcast([P, HID]), op=ALU.add)
                    nc.vector.tensor_scalar_max(h_sl[:, sl], tmp[:], 0.0)
                else:
                    nc.vector.tensor_scalar(
                        out=h_sl[:, sl], in0=ycb,
                        scalar1=isd_all[:, t:t + 1], scalar2=0.0,
                        op0=ALU.mult, op1=ALU.max)
            if l < L - 1:
                nc.sync.dma_start(
                    out=t_h[g0:g0 + gsz].rearrange("g p f -> p g f"),
                    in_=h_sl[:, :gsz * HID])
            else:
                hf_sl = lp.tile([P, GT * HID], fp8, tag="hf8")
                nc.scalar.copy(out=hf_sl[:, :gsz * HID],
                               in_=h_sl[:, :gsz * HID])
                nc.sync.dma_start(
                    out=t_hfin[g0 * P:(g0 + gsz) * P, :].rearrange(
                        "(g p) f -> p g f", p=P),
                    in_=hf_sl[:, :gsz * HID])

        def edge_phase(l):
            sub_tile = pr["sub_tile"]
            sub_start = pr["sub_start"]
            sub_stop = pr["sub_stop"]
            agg_map = {}
            gen = not (g_one[l] and b_zero[l])
            lng = lnb = None
            if gen:
                lng = cpool.tile([1, HID], f32, name=f"lng{l}")
                lnb = cpool.tile([1, HID], f32, name=f"lnb{l}")
            group_of_last = {g0 + gsz - 1: (g0, gsz)
                             for (g0, gsz) in groups}
            for b in range(NB):
                idx = ep.tile([P, KB], i32, tag="idx")
                nc.sync.dma_start(out=idx[:], in_=t_kvi[b])
                oh = ep.tile([P, KB * P], bf16, tag="oh")
                nc.sync.dma_start(out=oh[:], in_=t_oh[b])
                oht = ep.tile([P, KB * P], bf16, tag="oht")
                nc.scalar.dma_start(out=oht[:], in_=t_oht[b])
                oht3 = oht[:].rearrange("p (k n) -> p k n", n=P)
                kvg = ep.tile([P, KB * 2 * HID], bf16, tag="kvg")
                kvg3 = kvg[:].rearrange("p (k f) -> p k f", f=2 * HID)
                qg = ep.tile([P, KB * HID], bf16, tag="qg")
                for j in range(KB):
                    nc.gpsimd.indirect_dma_start(
                        out=kvg3[:, j, :], out_offset=None, in_=t_kv[:, :],
                        in_offset=bass.IndirectOffsetOnAxis(
                            ap=idx[:, j:j + 1], axis=0))
                for j0 in range(0, KB, QS):
                    qs_ps = pp_qs.tile([P, QS * HID], f32, tag="qs")
                    for j in range(j0, j0 + QS):
                        st = b * KB + j
                        t = int(sub_tile[st])
                        nc.tensor.matmul(
                            out=qs_ps[:, (j - j0) * HID:(j - j0 + 1) * HID],
                            lhsT=oht3[:, j, :],
                            rhs=q_all[:, t * HID:(t + 1) * HID],
                            start=True, stop=True)
                    nc.scalar.copy(
                        out=qg[:, j0 * HID:(j0 + QS) * HID], in_=qs_ps[:])
                prod = ep.tile([P, KB * HID], bf16, tag="prod")
                nc.vector.tensor_tensor(
                    out=prod[:].rearrange("p (k f) -> p k f", f=HID),
                    in0=kvg3[:, :, 0:HID],
                    in1=qg[:].rearrange("p (k f) -> p k f", f=HID),
                    op=ALU.mult)
                sc = ep.tile([P, KB * HEADS], f32, tag="sc")
                nc.vector.reduce_sum(
                    sc[:], prod[:].rearrange("p (g d) -> p g d", d=DH),
                    axis=X)
                ext = ep.tile([P, KB * HID], bf16, tag="ext")
                nc.scalar.activation(
                    out=ext[:].rearrange("p (k h d) -> p k h d", h=HEADS,
                                         d=DH),
                    in_=sc[:].rearrange("p (k h) -> p k h", h=HEADS)
                        .unsqueeze(-1).to_broadcast([P, KB, HEADS, DH]),
                    func=ACT.Exp, scale=scale)
                rhs = ep.tile([P, KB * (HID + HEADS)], bf16, tag="rhs")
                rhs3 = rhs[:].rearrange("p (k f) -> p k f", f=HID + HEADS)
                nc.scalar.activation(
                    out=rhs3[:, :, HID:HID + HEADS],
                    in_=sc[:].rearrange("p (k h) -> p k h", h=HEADS),
                    func=ACT.Exp, scale=scale)
                nc.vector.tensor_tensor(
                    out=rhs3[:, :, 0:HID], in0=kvg3[:, :, HID:2 * HID],
                    in1=ext[:].rearrange("p (k f) -> p k f", f=HID),
                    op=ALU.mult)
                oh3 = oh[:].rearrange("p (k n) -> p k n", n=P)
                for j in range(KB):
                    st = b * KB + j
                    t = int(sub_tile[st])
                    if sub_start[st]:
                        agg_map[t] = pp_agg.tile([P, HID + HEADS], f32,
                                                 tag="agg", name="aggt")
                    nc.tensor.matmul(
                        out=agg_map[t][:], lhsT=oh3[:, j, :],
                        rhs=rhs3[:, j, :], start=bool(sub_start[st]),
                        stop=bool(sub_stop[st]))
                    if sub_stop[st]:
                        pass1(l, t, agg_map.pop(t)[:])
                        if t in group_of_last:
                            fg0, fgsz = group_of_last[t]
                            finish_group(l, fg0, fgsz, gen, lng, lnb)

        def finish_layer(l):
            sd = epi.tile([P, NT], f32, tag="sd")
            nc.scalar.activation(out=sd[:], in_=var_all[:], func=ACT.Sqrt)
            nc.vector.reciprocal(isd_all[:], sd[:])
            gen = not (g_one[l] and b_zero[l])
            if gen:
                lng = cpool.tile([1, HID], f32, name=f"lng{l}")
                lnb = cpool.tile([1, HID], f32, name=f"lnb{l}")
            for (g0, gsz) in groups:
                h_sl = lp.tile([P, GT * HID], bf16, tag="ho")
                for j in range(gsz):
                    t = g0 + j
                    sl = slice(j * HID, (j + 1) * HID)
                    ycb = yc_all[:, t * HID:(t + 1) * HID]
                    if gen:
                        tmp = epi.tile([P, HID], bf16, tag="tmp")
                        nc.vector.tensor_scalar_mul(
                            tmp[:], ycb, isd_all[:, t:t + 1])
                        nc.vector.tensor_tensor(
                            out=tmp[:], in0=tmp[:],
                            in1=lng[:].to_broadcast([P, HID]), op=ALU.mult)
                        nc.vector.tensor_tensor(
                            out=tmp[:], in0=tmp[:],
                            in1=lnb[:].to_broadcast([P, HID]), op=ALU.add)
                        nc.vector.tensor_scalar_max(h_sl[:, sl], tmp[:], 0.0)
                    else:
                        nc.vector.tensor_scalar(
                            out=h_sl[:, sl], in0=ycb,
                            scalar1=isd_all[:, t:t + 1], scalar2=0.0,
                            op0=ALU.mult, op1=ALU.max)
                if l < L - 1:
                    nc.sync.dma_start(
                        out=t_h[g0:g0 + gsz].rearrange("g p f -> p g f"),
                        in_=h_sl[:, :gsz * HID])
                else:
                    nc.sync.dma_start(
                        out=t_hfin[g0 * P:(g0 + gsz) * P, :].rearrange(
                            "(g p) f -> p g f", p=P),
                        in_=h_sl[:, :gsz * HID])

        for l in range(L):
            linear_phase(l)
            nc.gpsimd.collective_compute(
                "AllGather", mybir.AluOpType.bypass, replica_groups=rg,
                ins=[t_kvin[:, :]], outs=[t_kv[:, :]])
            edge_phase(l)

        nc.gpsimd.collective_compute(
            "AllGather", mybir.AluOpType.bypass, replica_groups=rg,
            ins=[t_hfin[:, :]], outs=[t_hf[:, :]])

        for b in range(NPB):
            pidx = ep.tile([P, 2 * KBP], i32, tag="pidx")
            nc.sync.dma_start(out=pidx[:], in_=t_pidx[b])
            hs = ep.tile([P, KBP * HID], bf16, tag="hs")
            hs3 = hs[:].rearrange("p (k f) -> p k f", f=HID)
            hd = ep.tile([P, KBP * HID], bf16, tag="hd")
            hd3 = hd[:].rearrange("p (k f) -> p k f", f=HID)
            for j in range(KBP):
                nc.gpsimd.indirect_dma_start(
                    out=hs3[:, j, :], out_offset=None, in_=t_hf[:, :],
                    in_offset=bass.IndirectOffsetOnAxis(
                        ap=pidx[:, j:j + 1], axis=0))
                nc.gpsimd.indirect_dma_start(
                    out=hd3[:, j, :], out_offset=None, in_=t_hf[:, :],
                    in_offset=bass.IndirectOffsetOnAxis(
                        ap=pidx[:, KBP + j:KBP + j + 1], axis=0))
            pm = ep.tile([P, KBP * HID], bf16, tag="pm")
            nc.vector.tensor_tensor(out=pm[:], in0=hs[:], in1=hd[:],
                                    op=ALU.mult)
            dots = ep.tile([P, KBP], f32, tag="dots")
            nc.vector.reduce_sum(
                dots[:], pm[:].rearrange("p (k f) -> p k f", f=HID), axis=X)
            osb = ep.tile([P, KBP], f32, tag="osb")
            nc.scalar.activation(out=osb[:], in_=dots[:], func=ACT.Sigmoid)
            nc.sync.dma_start(
                out=t_out[b * KBP:(b + 1) * KBP, :].rearrange("k p -> p k"),
                in_=osb[:])

    nc.finalize()
    return nc


def kernel(**inputs):
    from concourse.bass_utils import run_bass_kernel_spmd

    pr = _prep(inputs)
    nc = _build(pr)

    in_maps = []
    for c in range(NCORES):
        m = {
            "x_t": pr["xT"][c],
            "rw_t": pr["rwT"][c],
            "w1": pr["W1"],
            "w2": pr["W2"],
            "kvidx": pr["kvidx"][c],
            "onehot": pr["oh"][c],
            "onehot_t": pr["oht"][c],
            "pidx": pr["pidx"][c],
        }
        for l in range(L):
            m[f"wc{l}"] = pr["Wcat"][l]
        if bool(np.any(pr["bcat"] != 0)):
            m["bc_r"] = pr["bcat"][None, :].astype(np.float32)
            m["bc_c"] = pr["bcat"][:, None].astype(np.float32)
        for l in range(L):
            if bool(np.any(pr["bqkvs"][l] != 0)):
                m[f"bqk{l}"] = pr["bqkvs"][l][None, :].astype(np.float32)
            if not (bool(np.all(pr["ln_g"][l] == 1))
                    and bool(np.all(pr["ln_b"][l] == 0))):
                m[f"lng{l}"] = pr["ln_g"][l][None, :].astype(np.float32)
                m[f"lnb{l}"] = pr["ln_b"][l][None, :].astype(np.float32)
        in_maps.append(m)

    kw = {}
    if os.environ.get("KERNEL_TMPDIR"):
        kw["tmpdir"] = os.environ["KERNEL_TMPDIR"]
    res = run_bass_kernel_spmd(
        nc, in_maps, core_ids=list(range(NCORES)),
        trace=bool(int(os.environ.get("KERNEL_TRACE", "0"))), **kw)
    if res.exec_time_ns is not None:
        print(f"HW exec time: {res.exec_time_ns} ns")

    PLOC = pr["PLOC"]
    outs = []
    for c in range(NCORES):
        vals = res.results[c]["out"].reshape(-1)  # [NPB*KBP*P]
        outs.append(vals[:pr["plocs"][c]])
    return np.concatenate(outs).astype(np.float32)



# revision 6
# speedup vs baseline: 1.0295x; 1.0295x over previous
"""Trainium2 Bass kernel for GraphTransformerLinkPredictor — v4.

v3 (4.25ms) trace: GpSimd 79% busy, 2858us in 2480 DMA_INDIRECT at
~1152ns each (~8.2ns/descriptor of serialized Q7 SWDGE time — verified
against dma_gather which runs at the same per-descriptor rate; the
multi-offset INDIRECT1D form returns garbage on this image, HWDGE
indirect wedges the device). So ~2.9ms of Pool-engine gather work is a
hard floor; v4 packs it back-to-back:
  - deep edge pipelining (kvg/oh/oht pools 4-6 bufs) so the per-batch
    ~1.4us Pool sem-stalls vanish.
  - chunked AllGathers: kv (per node tile-group, group-major chunk row
    layout so each chunk is contiguous) overlap the node phase; same
    for the final hf table. Pool only sees a ~200ns doorbell per chunk.
  - cross-layer overlap: per-group q/sh slab pools (bufs=15, WAR
    distance 14 > 13 groups) + per-group t_h/kvin tensors let layer
    l+1's node matmuls interleave into layer l's edge-gather stream
    right after each group's LayerNorm finishes.
  - double-buffered t_kv per layer (edge-l0 gathers still read t_kv0
    while kv1 chunks land in t_kv1).
Pair phase stays 2 indirect gathers per 128 pairs (PE one-hot
alternatives all violate the SPMD shared-addressing constraint).
"""

import math
import os
from contextlib import ExitStack

import numpy as np

P = 128
HID = 128
HEADS = 4
DH = 32
L = 2
EPS = 1e-5
NCORES = 8
KB = 8    # edge subtiles per DVE/ACT batch
QS = 4    # qsel matmuls per PSUM bank (one copy per QS subtiles)
GT = 8    # node tiles per linear-phase group
KBP = 8   # pair subtiles per DVE batch


def _groups(nt, g):
    out = []
    t0 = 0
    while t0 < nt:
        out.append((t0, min(g, nt - t0)))
        t0 += min(g, nt - t0)
    return out


def _prep(inputs):
    import ml_dtypes

    bf = ml_dtypes.bfloat16

    x = np.ascontiguousarray(np.asarray(inputs["x"], dtype=np.float32))
    rw = np.ascontiguousarray(np.asarray(inputs["rw_diag"], dtype=np.float32))
    ei = np.asarray(inputs["edge_index"]).astype(np.int64)
    psrc = np.asarray(inputs["src"]).astype(np.int64)
    pdst = np.asarray(inputs["dst"]).astype(np.int64)

    N = x.shape[0]
    IN_C = x.shape[1]
    RWD = rw.shape[1]
    NT = math.ceil(N / (NCORES * P))
    NLOC = NT * P
    NPADT = NLOC * NCORES

    W_rwse = np.asarray(inputs["W_rwse"], np.float32)
    b_rwse = np.asarray(inputs["b_rwse"], np.float32)
    W_in = np.asarray(inputs["W_in"], np.float32)
    b_in = np.asarray(inputs["b_in"], np.float32)
    W1 = np.ascontiguousarray(W_in[:IN_C]).astype(bf)
    W2 = np.ascontiguousarray(W_rwse @ W_in[IN_C:]).astype(bf)
    bcat = (b_in + b_rwse @ W_in[IN_C:]).astype(np.float32)

    Wq = np.asarray(inputs["Wq"], np.float32)
    Wk = np.asarray(inputs["Wk"], np.float32)
    Wv = np.asarray(inputs["Wv"], np.float32)
    Ws = np.asarray(inputs["Ws"], np.float32)
    bq = np.asarray(inputs["bq"], np.float32)
    bk = np.asarray(inputs["bk"], np.float32)
    bv = np.asarray(inputs["bv"], np.float32)
    bs = np.asarray(inputs["bs"], np.float32)
    ln_g = np.asarray(inputs["ln_g"], np.float32)
    ln_b = np.asarray(inputs["ln_b"], np.float32)

    Wcat = [np.ascontiguousarray(np.concatenate(
        [Wq[l], Wk[l], Wv[l], Ws[l]], axis=1)).astype(bf) for l in range(L)]
    bqkvs = [np.concatenate([bq[l], bk[l], bv[l], bs[l]]) for l in range(L)]

    groups = _groups(NT, GT)
    NG = len(groups)
    # group-major chunk row layout for the AllGathered tables:
    # row(n) = base[g] + core*gsz*P + (slot-g0)*P + p
    gbase = np.zeros(NG + 1, np.int64)
    for gi, (g0, gsz) in enumerate(groups):
        gbase[gi + 1] = gbase[gi] + NCORES * gsz * P

    row = ei[0]
    col = ei[1]
    core_of = col // NLOC
    tile_of = (col % NLOC) // P

    order = np.lexsort((row, tile_of, core_of))
    srow = row[order]
    scol = col[order]

    flat = core_of[order] * NT + tile_of[order]
    cnt = np.bincount(flat, minlength=NCORES * NT).reshape(NCORES, NT)
    # per-core local-tile permutation: slot s handles each core's s-th
    # biggest tile so the shared subtile counts align big-with-big.
    perm = np.argsort(-cnt, axis=1, kind="stable")      # [NC, NT] slot->tile
    inv_perm = np.argsort(perm, axis=1)                 # [NC, NT] tile->slot
    cnt_s = np.take_along_axis(cnt, perm, axis=1)       # counts per slot
    tcnt = np.maximum(1, np.ceil(cnt_s / P).astype(np.int64)).max(axis=0)
    total_t = int(tcnt.sum())
    tcnt[-1] += (-total_t) % KB
    ET = int(tcnt.sum())
    NB = ET // KB

    sgrp = np.arange(NT) // GT                          # slot -> group
    g0s = np.array([g0 for (g0, gsz) in groups])
    gszs = np.array([gsz for (g0, gsz) in groups])

    def slot_row(n):
        c = n // NLOC
        lo = n % NLOC
        s = inv_perm[c, lo // P]
        g = sgrp[s]
        return (gbase[g] + c * gszs[g] * P + (s - g0s[g]) * P + lo % P)

    sub_tile = np.repeat(np.arange(NT), tcnt)
    starts = np.concatenate([[0], np.cumsum(tcnt)])[:-1] * P
    sub_start = np.zeros(ET, bool)
    sub_stop = np.zeros(ET, bool)
    sub_start[starts // P] = True
    sub_stop[(starts // P + tcnt - 1)] = True

    msrow = slot_row(srow)
    kvidx = np.zeros((NCORES, ET * P), np.int32)
    ohm = np.zeros((NCORES, ET * P, P), bf)
    ohtm = np.zeros((NCORES, ET, P, P), bf)  # [sub, node, edge]
    gstart = np.concatenate([[0], np.cumsum(cnt.reshape(-1))])
    for c in range(NCORES):
        for s in range(NT):
            t = perm[c, s]
            g0 = gstart[c * NT + t]
            n = cnt[c, t]
            if n == 0:
                continue
            o = starts[s]
            sl = np.arange(o, o + n)
            kvidx[c, sl] = msrow[g0:g0 + n]
            cl = (scol[g0:g0 + n] - c * NLOC).astype(np.int32)
            ohm[c, sl, cl % P] = 1
            ohtm[c, sl // P, cl % P, sl % P] = 1

    kvidx_hw = kvidx.reshape(NCORES, NB, KB, P).transpose(0, 1, 3, 2)
    kvidx_hw = np.ascontiguousarray(kvidx_hw)
    oh_hw = np.ascontiguousarray(
        ohm.reshape(NCORES, NB, KB, P, P).transpose(0, 1, 3, 2, 4)).reshape(
        NCORES, NB, P, KB * P)
    oht_hw = np.ascontiguousarray(
        ohtm.reshape(NCORES, NB, KB, P, P).transpose(0, 1, 3, 2, 4)).reshape(
        NCORES, NB, P, KB * P)

    # node features (zero-padded, transposed tiles, slot order)
    xs = np.zeros((NCORES, NLOC, IN_C), np.float32)
    rws = np.zeros((NCORES, NLOC, RWD), np.float32)
    for c in range(NCORES):
        lo = c * NLOC
        hi = min(N, lo + NLOC)
        if hi > lo:
            xs[c, :hi - lo] = x[lo:hi]
            rws[c, :hi - lo] = rw[lo:hi]
    ci = np.arange(NCORES)[:, None]
    xT_hw = np.ascontiguousarray(
        xs.reshape(NCORES, NT, P, IN_C)[ci, perm].transpose(
            0, 1, 3, 2)).astype(bf)
    rwT_hw = np.ascontiguousarray(
        rws.reshape(NCORES, NT, P, RWD)[ci, perm].transpose(
            0, 1, 3, 2)).astype(bf)

    # pairs: contiguous shards per core
    NPAIR = psrc.shape[0]
    PLOC = math.ceil(NPAIR / NCORES)
    plocs = [max(0, min(PLOC, NPAIR - c * PLOC)) for c in range(NCORES)]
    NPS = math.ceil(PLOC / P)
    NPB = math.ceil(NPS / KBP)
    PPAD = NPB * KBP * P
    ps = np.zeros((NCORES, PPAD), np.int32)
    pd = np.zeros((NCORES, PPAD), np.int32)
    mpsrc = slot_row(psrc)
    mpdst = slot_row(pdst)
    for c in range(NCORES):
        n = plocs[c]
        ps[c, :n] = mpsrc[c * PLOC:c * PLOC + n]
        pd[c, :n] = mpdst[c * PLOC:c * PLOC + n]
    pidx_hw = np.zeros((NCORES, NPB, P, 2 * KBP), np.int32)
    pidx_hw[:, :, :, 0:KBP] = ps.reshape(
        NCORES, NPB, KBP, P).transpose(0, 1, 3, 2)
    pidx_hw[:, :, :, KBP:] = pd.reshape(
        NCORES, NPB, KBP, P).transpose(0, 1, 3, 2)

    return dict(
        N=N, IN_C=IN_C, RWD=RWD, NT=NT, NLOC=NLOC, NPADT=NPADT,
        ET=ET, NB=NB, NPB=NPB, PLOC=PLOC, plocs=plocs, NPAIR=NPAIR,
        NG=NG, groups=groups, gbase=gbase,
        W1=W1, W2=W2, bcat=bcat, Wcat=Wcat, bqkvs=bqkvs,
        ln_g=ln_g, ln_b=ln_b,
        xT=xT_hw, rwT=rwT_hw, kvidx=kvidx_hw, oh=oh_hw, oht=oht_hw,
        pidx=pidx_hw,
        sub_tile=sub_tile, sub_start=sub_start, sub_stop=sub_stop,
    )


def _build(pr):
    import concourse.bass as bass
    import concourse.bacc as bacc
    import concourse.mybir as mybir
    import concourse.tile as tile

    f32 = mybir.dt.float32
    bf16 = mybir.dt.bfloat16
    fp8 = mybir.dt.float8e4
    i32 = mybir.dt.int32
    ALU = mybir.AluOpType
    ACT = mybir.ActivationFunctionType
    X = mybir.AxisListType.X

    NT, NLOC, NPADT = pr["NT"], pr["NLOC"], pr["NPADT"]
    NB, NPB = pr["NB"], pr["NPB"]
    IN_C, RWD = pr["IN_C"], pr["RWD"]
    NG, groups, gbase = pr["NG"], pr["groups"], pr["gbase"]
    scale = 1.0 / math.sqrt(DH)

    bias_nz = bool(np.any(pr["bcat"] != 0))
    qkvs_nz = [bool(np.any(b != 0)) for b in pr["bqkvs"]]
    g_one = [bool(np.all(pr["ln_g"][l] == 1)) for l in range(L)]
    b_zero = [bool(np.all(pr["ln_b"][l] == 0)) for l in range(L)]

    nc = bacc.Bacc(None, num_devices=NCORES)

    t_xT = nc.dram_tensor("x_t", [NT, IN_C, P], bf16, kind="ExternalInput")
    t_rwT = nc.dram_tensor("rw_t", [NT, RWD, P], bf16, kind="ExternalInput")
    t_w1 = nc.dram_tensor("w1", [IN_C, HID], bf16, kind="ExternalInput")
    t_w2 = nc.dram_tensor("w2", [RWD, HID], bf16, kind="ExternalInput")
    t_wc = [nc.dram_tensor(f"wc{l}", [HID, 4 * HID], bf16,
                           kind="ExternalInput") for l in range(L)]
    t_kvi = nc.dram_tensor("kvidx", [NB, P, KB], i32, kind="ExternalInput")
    t_oh = nc.dram_tensor("onehot", [NB, P, KB * P], bf16,
                          kind="ExternalInput")
    t_oht = nc.dram_tensor("onehot_t", [NB, P, KB * P], bf16,
                           kind="ExternalInput")
    t_pidx = nc.dram_tensor("pidx", [NPB, P, 2 * KBP], i32,
                            kind="ExternalInput")

    # per-group inter-layer h and per-(layer, group) kv inputs; chunked
    # AllGather destinations are double-buffered per layer.
    t_h = [nc.dram_tensor(f"h_g{g}", [gsz, P, HID], bf16, kind="Internal")
           for g, (g0, gsz) in enumerate(groups)]
    t_kvin = [[nc.dram_tensor(f"kv_in{l}_g{g}", [gsz * P, 2 * HID], fp8,
                              kind="Internal")
               for g, (g0, gsz) in enumerate(groups)] for l in range(L)]
    t_kv = [nc.dram_tensor(f"kv_full{l}", [NPADT, 2 * HID], fp8,
                           kind="Internal", addr_space="Shared")
            for l in range(L)]
    t_hfin = [nc.dram_tensor(f"hf_in_g{g}", [gsz * P, HID], fp8,
                             kind="Internal")
              for g, (g0, gsz) in enumerate(groups)]
    t_hf = nc.dram_tensor("hf_full", [NPADT, HID], fp8, kind="Internal",
                          addr_space="Shared")
    t_out = nc.dram_tensor("out", [NPB * KBP, P], f32,
                           kind="ExternalOutput")

    rg = [list(range(NCORES))]
    g_of_slot = {s: gi for gi, (g0, gsz) in enumerate(groups)
                 for s in range(g0, g0 + gsz)}
    last_of_group = {g0 + gsz - 1: gi
                     for gi, (g0, gsz) in enumerate(groups)}

    with ExitStack() as ctx:
        tc = ctx.enter_context(tile.TileContext(nc))
        cpool = ctx.enter_context(tc.tile_pool(name="const", bufs=1))
        qpool = ctx.enter_context(tc.tile_pool(name="qp", bufs=15))
        shpool = ctx.enter_context(tc.tile_pool(name="shp", bufs=15))
        ycpool = ctx.enter_context(tc.tile_pool(name="ycp", bufs=3))
        lp = ctx.enter_context(tc.tile_pool(name="lp", bufs=2))
        ep = ctx.enter_context(tc.tile_pool(name="ep", bufs=4))
        kp = ctx.enter_context(tc.tile_pool(name="kp", bufs=6))
        epi = ctx.enter_context(tc.tile_pool(name="epi", bufs=2))
        pp_mm = ctx.enter_context(tc.tile_pool(name="ppm", bufs=2,
                                               space="PSUM"))
        pp_agg = ctx.enter_context(tc.tile_pool(name="ppa", bufs=3,
                                                space="PSUM"))
        pp_qs = ctx.enter_context(tc.tile_pool(name="ppq", bufs=2,
                                               space="PSUM"))

        w1 = cpool.tile([IN_C, HID], bf16)
        nc.sync.dma_start(out=w1[:], in_=t_w1[:, :])
        w2 = cpool.tile([RWD, HID], bf16)
        nc.sync.dma_start(out=w2[:], in_=t_w2[:, :])
        wc = []
        for l in range(L):
            w = cpool.tile([HID, 4 * HID], bf16, name=f"wc{l}")
            nc.sync.dma_start(out=w[:], in_=t_wc[l][:, :])
            wc.append(w)

        # per-group slabs, keyed (layer, group)
        q_sl = {}
        sh_sl = {}
        yc_sl = {}
        var_sl = {}

        def node_group(l, gi):
            g0, gsz = groups[gi]
            q_sl[(l, gi)] = qpool.tile([P, GT * HID], bf16, tag="q",
                                        name="qsl")
            sh_sl[(l, gi)] = shpool.tile([P, GT * HID], bf16, tag="sh",
                                          name="shsl")
            hT_sl = lp.tile([P, GT * P], bf16, tag="hT")
            h_sl = lp.tile([P, GT * P], bf16, tag="h")
            if l == 0:
                xT_sl = lp.tile([P, GT * P], bf16, tag="xT")
                nc.sync.dma_start(
                    out=xT_sl[:, :gsz * P],
                    in_=t_xT[g0:g0 + gsz].rearrange("g c n -> c g n"))
                rwT_sl = lp.tile([RWD, GT * P], bf16, tag="rwT")
                nc.sync.dma_start(
                    out=rwT_sl[:, :gsz * P],
                    in_=t_rwT[g0:g0 + gsz].rearrange("g c n -> c g n"))
                for j in range(gsz):
                    sl = slice(j * P, (j + 1) * P)
                    hT_ps = pp_mm.tile([P, P], f32, tag="mm0")
                    nc.tensor.matmul(out=hT_ps[:], lhsT=w1[:],
                                     rhs=xT_sl[:, sl], start=True,
                                     stop=False)
                    nc.tensor.matmul(out=hT_ps[:], lhsT=w2[:],
                                     rhs=rwT_sl[:, sl], start=False,
                                     stop=True)
                    h_ps = pp_mm.tile([P, P], f32, tag="mm0")
                    nc.tensor.matmul(out=h_ps[:], lhsT=xT_sl[:, sl],
                                     rhs=w1[:], start=True, stop=False)
                    nc.tensor.matmul(out=h_ps[:], lhsT=rwT_sl[:, sl],
                                     rhs=w2[:], start=False, stop=True)
                    nc.scalar.copy(out=hT_sl[:, sl], in_=hT_ps[:])
                    nc.vector.tensor_copy(h_sl[:, sl], h_ps[:])
                if bias_nz:
                    bc_r = cpool.tile([1, HID], f32, name="bc_r")
                    bc_c = cpool.tile([P, 1], f32, name="bc_c")
                    nc.vector.tensor_tensor(
                        out=h_sl[:, :gsz * P].rearrange(
                            "p (g f) -> p g f", f=HID),
                        in0=h_sl[:, :gsz * P].rearrange(
                            "p (g f) -> p g f", f=HID),
                        in1=bc_r[:].to_broadcast([P, gsz, HID]),
                        op=ALU.add)
                    nc.vector.tensor_scalar_add(
                        hT_sl[:, :gsz * P], hT_sl[:, :gsz * P], bc_c[:])
            else:
                nc.sync.dma_start(
                    out=hT_sl[:, :gsz * P],
                    in_=t_h[gi].rearrange("g p f -> (g p) f"),
                    transpose=True)
                nc.scalar.dma_start(
                    out=h_sl[:, :gsz * P],
                    in_=t_h[gi].rearrange("g p f -> p g f"))
            kvs_sl = lp.tile([P, GT * 2 * HID], fp8, tag="kvs")
            for j in range(gsz):
                t = g0 + j
                sl = slice(j * P, (j + 1) * P)
                qk_ps = pp_mm.tile([P, 4 * HID], f32, tag="mm0")
                nc.tensor.matmul(out=qk_ps[:], lhsT=hT_sl[:, sl],
                                 rhs=wc[l][:], start=True, stop=True)
                if qkvs_nz[l]:
                    bqk = cpool.tile([1, 4 * HID], f32, name=f"bqk{l}")
                    nc.vector.tensor_tensor(
                        out=qk_ps[:], in0=qk_ps[:],
                        in1=bqk[:].to_broadcast([P, 4 * HID]), op=ALU.add)
                nc.scalar.copy(out=q_sl[(l, gi)][:, j * HID:(j + 1) * HID],
                               in_=qk_ps[:, 0:HID])
                nc.scalar.copy(
                    out=kvs_sl[:, j * 2 * HID:(j + 1) * 2 * HID],
                    in_=qk_ps[:, HID:3 * HID])
                nc.vector.tensor_tensor(
                    out=sh_sl[(l, gi)][:, j * HID:(j + 1) * HID],
                    in0=qk_ps[:, 3 * HID:4 * HID], in1=h_sl[:, sl],
                    op=ALU.add)
            nc.sync.dma_start(
                out=t_kvin[l][gi][:, :].rearrange(
                    "(g p) f -> p g f", p=P),
                in_=kvs_sl[:, :gsz * 2 * HID])

        def ag_chunk_kv(l, gi):
            g0, gsz = groups[gi]
            nc.gpsimd.collective_compute(
                "AllGather", mybir.AluOpType.bypass, replica_groups=rg,
                ins=[t_kvin[l][gi][:, :]],
                outs=[t_kv[l][gbase[gi]:gbase[gi + 1], :]])

        def ag_chunk_hf(gi):
            nc.gpsimd.collective_compute(
                "AllGather", mybir.AluOpType.bypass, replica_groups=rg,
                ins=[t_hfin[gi][:, :]],
                outs=[t_hf[gbase[gi]:gbase[gi + 1], :]])

        def pass1(l, t, agg):
            gi = g_of_slot[t]
            g0, gsz = groups[gi]
            if (l, gi) not in yc_sl:
                yc_sl[(l, gi)] = ycpool.tile([P, GT * HID], bf16, tag="yc",
                                          name="ycsl")
                var_sl[(l, gi)] = ycpool.tile([P, GT], f32, tag="var",
                                           name="varsl")
            zsb = epi.tile([P, HEADS], f32, tag="zsb")
            nc.vector.tensor_scalar_max(zsb[:], agg[:, HID:HID + HEADS],
                                        1e-30)
            inv = epi.tile([P, HEADS], f32, tag="inv")
            nc.vector.reciprocal(inv[:], zsb[:])
            y = epi.tile([P, HID], bf16, tag="y")
            nc.vector.tensor_tensor(
                out=y[:].rearrange("p (h d) -> p h d", d=DH),
                in0=agg[:, 0:HID].rearrange("p (h d) -> p h d", d=DH),
                in1=inv[:].unsqueeze(-1).to_broadcast([P, HEADS, DH]),
                op=ALU.mult)
            nc.vector.tensor_tensor(
                out=y[:], in0=y[:],
                in1=sh_sl[(l, gi)][:, (t - g0) * HID:(t - g0 + 1) * HID],
                op=ALU.add)
            musum = epi.tile([P, 1], f32, tag="musum")
            nc.vector.reduce_sum(musum[:], y[:], axis=X)
            mu = epi.tile([P, 1], f32, tag="mu")
            nc.vector.tensor_scalar_mul(mu[:], musum[:], 1.0 / HID)
            yc = yc_sl[(l, gi)][:, (t - g0) * HID:(t - g0 + 1) * HID]
            nc.vector.tensor_scalar_sub(yc, y[:], mu[:])
            junk = epi.tile([P, HID], bf16, tag="junk")
            ssq = epi.tile([P, 1], f32, tag="ssq")
            nc.vector.scalar_tensor_tensor(
                out=junk[:], in0=yc, scalar=1.0, in1=yc,
                op0=ALU.bypass, op1=ALU.mult, accum_out=ssq[:])
            nc.vector.tensor_scalar(
                out=var_sl[(l, gi)][:, (t - g0):(t - g0 + 1)], in0=ssq[:],
                scalar1=1.0 / HID, scalar2=EPS, op0=ALU.mult, op1=ALU.add)

        def finish_group(l, gi, gen, lng, lnb):
            g0, gsz = groups[gi]
            ycb_all = yc_sl.pop((l, gi))
            varb = var_sl.pop((l, gi))
            sdg = epi.tile([P, GT], f32, tag="sdg")
            nc.scalar.activation(out=sdg[:, :gsz], in_=varb[:, :gsz],
                                 func=ACT.Sqrt)
            isd = epi.tile([P, GT], f32, tag="isd")
            nc.vector.reciprocal(isd[:, :gsz], sdg[:, :gsz])
            h_sl = lp.tile([P, GT * HID], bf16, tag="ho")
            for j in range(gsz):
                sl = slice(j * HID, (j + 1) * HID)
                ycb = ycb_all[:, sl]
                if gen:
                    tmp = epi.tile([P, HID], bf16, tag="tmp")
                    nc.vector.tensor_scalar_mul(
                        tmp[:], ycb, isd[:, j:j + 1])
                    nc.vector.tensor_tensor(
                        out=tmp[:], in0=tmp[:],
                        in1=lng[:].to_broadcast([P, HID]), op=ALU.mult)
                    nc.vector.tensor_tensor(
                        out=tmp[:], in0=tmp[:],
                        in1=lnb[:].to_broadcast([P, HID]), op=ALU.add)
                    nc.vector.tensor_scalar_max(h_sl[:, sl], tmp[:], 0.0)
                else:
                    nc.vector.tensor_scalar(
                        out=h_sl[:, sl], in0=ycb,
                        scalar1=isd[:, j:j + 1], scalar2=0.0,
                        op0=ALU.mult, op1=ALU.max)
            if l < L - 1:
                nc.sync.dma_start(
                    out=t_h[gi].rearrange("g p f -> p g f"),
                    in_=h_sl[:, :gsz * HID])
            else:
                hf_sl = lp.tile([P, GT * HID], fp8, tag="hf8")
                nc.scalar.copy(out=hf_sl[:, :gsz * HID],
                               in_=h_sl[:, :gsz * HID])
                nc.sync.dma_start(
                    out=t_hfin[gi][:, :].rearrange("(g p) f -> p g f", p=P),
                    in_=hf_sl[:, :gsz * HID])

        def edge_batch(l, b, agg_map, gen, lng, lnb, finish_hook,
                       pending):
            # drain queued chunk-collective emissions whose delay expired
            # (delayed so the Pool-side CC doorbell never waits on a DMA
            # that PE hasn't produced yet)
            while pending and pending[0][0] <= b:
                pending.pop(0)[1]()
            sub_tile = pr["sub_tile"]
            sub_start = pr["sub_start"]
            sub_stop = pr["sub_stop"]
            idx = ep.tile([P, KB], i32, tag="idx")
            nc.sync.dma_start(out=idx[:], in_=t_kvi[b])
            oh = ep.tile([P, KB * P], bf16, tag="oh")
            nc.sync.dma_start(out=oh[:], in_=t_oh[b])
            oht = ep.tile([P, KB * P], bf16, tag="oht")
            nc.scalar.dma_start(out=oht[:], in_=t_oht[b])
            oht3 = oht[:].rearrange("p (k n) -> p k n", n=P)
            kvg = kp.tile([P, KB * 2 * HID], bf16, tag="kvg")
            kvg3 = kvg[:].rearrange("p (k f) -> p k f", f=2 * HID)
            qg = ep.tile([P, KB * HID], bf16, tag="qg")
            for j in range(KB):
                nc.gpsimd.indirect_dma_start(
                    out=kvg3[:, j, :], out_offset=None, in_=t_kv[l][:, :],
                    in_offset=bass.IndirectOffsetOnAxis(
                        ap=idx[:, j:j + 1], axis=0))
            for j0 in range(0, KB, QS):
                qs_ps = pp_qs.tile([P, QS * HID], f32, tag="qs")
                for j in range(j0, j0 + QS):
                    st = b * KB + j
                    t = int(sub_tile[st])
                    gi = g_of_slot[t]
                    g0 = groups[gi][0]
                    nc.tensor.matmul(
                        out=qs_ps[:, (j - j0) * HID:(j - j0 + 1) * HID],
                        lhsT=oht3[:, j, :],
                        rhs=q_sl[(l, gi)][:, (t - g0) * HID:
                                          (t - g0 + 1) * HID],
                        start=True, stop=True)
                nc.scalar.copy(
                    out=qg[:, j0 * HID:(j0 + QS) * HID], in_=qs_ps[:])
            prod = ep.tile([P, KB * HID], bf16, tag="prod")
            nc.vector.tensor_tensor(
                out=prod[:].rearrange("p (k f) -> p k f", f=HID),
                in0=kvg3[:, :, 0:HID],
                in1=qg[:].rearrange("p (k f) -> p k f", f=HID),
                op=ALU.mult)
            sc = ep.tile([P, KB * HEADS], f32, tag="sc")
            nc.vector.reduce_sum(
                sc[:], prod[:].rearrange("p (g d) -> p g d", d=DH),
                axis=X)
            ext = ep.tile([P, KB * HID], bf16, tag="ext")
            nc.scalar.activation(
                out=ext[:].rearrange("p (k h d) -> p k h d", h=HEADS,
                                     d=DH),
                in_=sc[:].rearrange("p (k h) -> p k h", h=HEADS)
                    .unsqueeze(-1).to_broadcast([P, KB, HEADS, DH]),
                func=ACT.Exp, scale=scale)
            rhs = ep.tile([P, KB * (HID + HEADS)], bf16, tag="rhs")
            rhs3 = rhs[:].rearrange("p (k f) -> p k f", f=HID + HEADS)
            nc.scalar.activation(
                out=rhs3[:, :, HID:HID + HEADS],
                in_=sc[:].rearrange("p (k h) -> p k h", h=HEADS),
                func=ACT.Exp, scale=scale)
            nc.vector.tensor_tensor(
                out=rhs3[:, :, 0:HID], in0=kvg3[:, :, HID:2 * HID],
                in1=ext[:].rearrange("p (k f) -> p k f", f=HID),
                op=ALU.mult)
            oh3 = oh[:].rearrange("p (k n) -> p k n", n=P)
            for j in range(KB):
                st = b * KB + j
                t = int(sub_tile[st])
                if sub_start[st]:
                    agg_map[t] = pp_agg.tile([P, HID + HEADS], f32,
                                             tag="agg", name="aggt")
                nc.tensor.matmul(
                    out=agg_map[t][:], lhsT=oh3[:, j, :],
                    rhs=rhs3[:, j, :], start=bool(sub_start[st]),
                    stop=bool(sub_stop[st]))
                if sub_stop[st]:
                    pass1(l, t, agg_map.pop(t)[:])
                    if t in last_of_group:
                        gi = last_of_group[t]
                        finish_group(l, gi, gen, lng, lnb)
                        finish_hook(gi, b, pending)

        def edge_phase(l, finish_hook):
            agg_map = {}
            pending = []
            gen = not (g_one[l] and b_zero[l])
            lng = lnb = None
            if gen:
                lng = cpool.tile([1, HID], f32, name=f"lng{l}")
                lnb = cpool.tile([1, HID], f32, name=f"lnb{l}")
            for b in range(NB):
                edge_batch(l, b, agg_map, gen, lng, lnb, finish_hook,
                           pending)
            for _, fn in pending:
                fn()

        # ---- emission schedule ----
        for gi in range(NG):
            node_group(0, gi)
            ag_chunk_kv(0, gi)

        def hook_l0(gi, b, pending):
            node_group(1, gi)
            pending.append((b + 3, lambda gi=gi: ag_chunk_kv(1, gi)))

        def hook_l1(gi, b, pending):
            pending.append((b + 3, lambda gi=gi: ag_chunk_hf(gi)))

        edge_phase(0, hook_l0)
        edge_phase(1, hook_l1)

        for b in range(NPB):
            pidx = ep.tile([P, 2 * KBP], i32, tag="pidx")
            nc.sync.dma_start(out=pidx[:], in_=t_pidx[b])
            hs = ep.tile([P, KBP * HID], bf16, tag="hs")
            hs3 = hs[:].rearrange("p (k f) -> p k f", f=HID)
            hd = ep.tile([P, KBP * HID], bf16, tag="hd")
            hd3 = hd[:].rearrange("p (k f) -> p k f", f=HID)
            for j in range(KBP):
                nc.gpsimd.indirect_dma_start(
                    out=hs3[:, j, :], out_offset=None, in_=t_hf[:, :],
                    in_offset=bass.IndirectOffsetOnAxis(
                        ap=pidx[:, j:j + 1], axis=0))
                nc.gpsimd.indirect_dma_start(
                    out=hd3[:, j, :], out_offset=None, in_=t_hf[:, :],
                    in_offset=bass.IndirectOffsetOnAxis(
                        ap=pidx[:, KBP + j:KBP + j + 1], axis=0))
            pm = ep.tile([P, KBP * HID], bf16, tag="pm")
            nc.vector.tensor_tensor(out=pm[:], in0=hs[:], in1=hd[:],
                                    op=ALU.mult)
            dots = ep.tile([P, KBP], f32, tag="dots")
            nc.vector.reduce_sum(
                dots[:], pm[:].rearrange("p (k f) -> p k f", f=HID), axis=X)
            osb = ep.tile([P, KBP], f32, tag="osb")
            nc.scalar.activation(out=osb[:], in_=dots[:], func=ACT.Sigmoid)
            nc.sync.dma_start(
                out=t_out[b * KBP:(b + 1) * KBP, :].rearrange("k p -> p k"),
                in_=osb[:])

    nc.finalize()
    return nc


def kernel(**inputs):
    from concourse.bass_utils import run_bass_kernel_spmd

    pr = _prep(inputs)
    nc = _build(pr)

    in_maps = []
    for c in range(NCORES):
        m = {
            "x_t": pr["xT"][c],
            "rw_t": pr["rwT"][c],
            "w1": pr["W1"],
            "w2": pr["W2"],
            "kvidx": pr["kvidx"][c],
            "onehot": pr["oh"][c],
            "onehot_t": pr["oht"][c],
            "pidx": pr["pidx"][c],
        }
        for l in range(L):
            m[f"wc{l}"] = pr["Wcat"][l]
        if bool(np.any(pr["bcat"] != 0)):
            m["bc_r"] = pr["bcat"][None, :].astype(np.float32)
            m["bc_c"] = pr["bcat"][:, None].astype(np.float32)
        for l in range(L):
            if bool(np.any(pr["bqkvs"][l] != 0)):
                m[f"bqk{l}"] = pr["bqkvs"][l][None, :].astype(np.float32)
            if not (bool(np.all(pr["ln_g"][l] == 1))
                    and bool(np.all(pr["ln_b"][l] == 0))):
                m[f"lng{l}"] = pr["ln_g"][l][None, :].astype(np.float32)
                m[f"lnb{l}"] = pr["ln_b"][l][None, :].astype(np.float32)
        in_maps.append(m)

    kw = {}
    if os.environ.get("KERNEL_TMPDIR"):
        kw["tmpdir"] = os.environ["KERNEL_TMPDIR"]
    res = run_bass_kernel_spmd(
        nc, in_maps, core_ids=list(range(NCORES)),
        trace=bool(int(os.environ.get("KERNEL_TRACE", "0"))), **kw)
    if res.exec_time_ns is not None:
        print(f"HW exec time: {res.exec_time_ns} ns")

    outs = []
    for c in range(NCORES):
        vals = res.results[c]["out"].reshape(-1)  # [NPB*KBP*P]
        outs.append(vals[:pr["plocs"][c]])
    return np.concatenate(outs).astype(np.float32)


# revision 8
# speedup vs baseline: 1.3468x; 1.3082x over previous
"""Trainium2 Bass kernel for GraphTransformerLinkPredictor — v5.

v4 (4.14ms) trace: Pool serialized on 2480 DMA_INDIRECT x (1104ns Q7 +
309ns sequencer dispatch) = 11ns/row of Pool time. v5 switches every
data-dependent move to InstDMAGatherAnt (dma_gather):
  - ~8.4ns/descriptor Q7 loop but ONE dispatch per up-to-1024 rows;
  - queue_num q runs on Q7 core pair q -> with num_swdge_queues=4 and
    rotating queues, measured 1.83x overlap => ~4.6ns/row Pool time.
Costs taken to get there:
  - int16 indices are sign-extended (15-bit): gathers read from one
    25088-row quarter of the table per instruction. Edge subtiles are
    (target-tile x quarter)-pure (+~14% padded subtiles); per-tile
    processing batches issue <=4 quarter-run gathers each.
  - no dtype cast in dma_gather: kv/hf tables are bf16 (2x AllGather
    bytes, still chunk-overlapped).
  - pairs sorted into 16 (src-quarter x dst-quarter) buckets, padded to
    a shared per-bucket subtile count across cores.
Keeps from v4: group-major chunked AllGathers overlapping producers,
per-group q/sh slab pools for cross-layer overlap, per-group t_h/kvin
tensors, inline pass1/finish hooks.
"""

import math
import os
from contextlib import ExitStack

import numpy as np

P = 128
HID = 128
HEADS = 4
DH = 32
L = 2
EPS = 1e-5
NCORES = 8
NQUART = 4
GT = 8    # node tiles per linear-phase group
NQ = 4    # SWDGE queues
MAXSUB = 8  # max subtiles per gather instruction
QS = 4    # qsel matmuls per PSUM tile


def _groups(nt, g):
    out = []
    t0 = 0
    while t0 < nt:
        out.append((t0, min(g, nt - t0)))
        t0 += min(g, nt - t0)
    return out


def _pack_idx16(vals):
    """[n*128] row indices -> dma_gather idx layout [128, n*8] int16
    (slot i at [i%16, i//16], replicated across the 8 core groups)."""
    n = vals.shape[0]
    sc = n // 16
    lay = vals.reshape(sc, 16).T.astype(np.int16)   # [16, SC]
    return np.tile(lay, (8, 1))                     # [128, SC]


def _prep(inputs):
    import ml_dtypes

    bf = ml_dtypes.bfloat16

    x = np.ascontiguousarray(np.asarray(inputs["x"], dtype=np.float32))
    rw = np.ascontiguousarray(np.asarray(inputs["rw_diag"], dtype=np.float32))
    ei = np.asarray(inputs["edge_index"]).astype(np.int64)
    psrc = np.asarray(inputs["src"]).astype(np.int64)
    pdst = np.asarray(inputs["dst"]).astype(np.int64)

    N = x.shape[0]
    IN_C = x.shape[1]
    RWD = rw.shape[1]
    NT = math.ceil(N / (NCORES * P))
    NLOC = NT * P
    NPADT = NLOC * NCORES
    QR = NPADT // NQUART            # rows per table quarter

    W_rwse = np.asarray(inputs["W_rwse"], np.float32)
    b_rwse = np.asarray(inputs["b_rwse"], np.float32)
    W_in = np.asarray(inputs["W_in"], np.float32)
    b_in = np.asarray(inputs["b_in"], np.float32)
    W1 = np.ascontiguousarray(W_in[:IN_C]).astype(bf)
    W2 = np.ascontiguousarray(W_rwse @ W_in[IN_C:]).astype(bf)
    bcat = (b_in + b_rwse @ W_in[IN_C:]).astype(np.float32)

    Wq = np.asarray(inputs["Wq"], np.float32)
    Wk = np.asarray(inputs["Wk"], np.float32)
    Wv = np.asarray(inputs["Wv"], np.float32)
    Ws = np.asarray(inputs["Ws"], np.float32)
    bq = np.asarray(inputs["bq"], np.float32)
    bk = np.asarray(inputs["bk"], np.float32)
    bv = np.asarray(inputs["bv"], np.float32)
    bs = np.asarray(inputs["bs"], np.float32)
    ln_g = np.asarray(inputs["ln_g"], np.float32)
    ln_b = np.asarray(inputs["ln_b"], np.float32)

    Wcat = [np.ascontiguousarray(np.concatenate(
        [Wq[l], Wk[l], Wv[l], Ws[l]], axis=1)).astype(bf) for l in range(L)]
    bqkvs = [np.concatenate([bq[l], bk[l], bv[l], bs[l]]) for l in range(L)]

    groups = _groups(NT, GT)
    NG = len(groups)
    gbase = np.zeros(NG + 1, np.int64)
    for gi, (g0, gsz) in enumerate(groups):
        gbase[gi + 1] = gbase[gi] + NCORES * gsz * P

    row = ei[0]
    col = ei[1]
    core_of = col // NLOC
    tile_of = (col % NLOC) // P

    # first pass: per-core per-tile counts to pick the slot permutation
    flat = core_of * NT + tile_of
    cnt = np.bincount(flat, minlength=NCORES * NT).reshape(NCORES, NT)
    perm = np.argsort(-cnt, axis=1, kind="stable")      # [NC, NT] slot->tile
    inv_perm = np.argsort(perm, axis=1)                 # [NC, NT] tile->slot

    sgrp = np.arange(NT) // GT
    g0s = np.array([g0 for (g0, gsz) in groups])
    gszs = np.array([gsz for (g0, gsz) in groups])

    def slot_row(n):
        c = n // NLOC
        lo = n % NLOC
        s = inv_perm[c, lo // P]
        g = sgrp[s]
        return (gbase[g] + c * gszs[g] * P + (s - g0s[g]) * P + lo % P)

    # second pass: sort edges by (core, slot, src-quarter, src-row)
    msrow_all = slot_row(row)
    quart = msrow_all // QR
    slot_of = np.take_along_axis(
        inv_perm[core_of], tile_of[None, :].T, axis=1).ravel() \
        if False else inv_perm[core_of, tile_of]
    order = np.lexsort((msrow_all, quart, slot_of, core_of))
    srow_m = msrow_all[order]
    scol = col[order]
    squart = quart[order]
    score_ = core_of[order]
    sslot = slot_of[order]

    # per (core, slot, quarter) counts -> shared subtile schedule
    cntq = np.zeros((NCORES, NT, NQUART), np.int64)
    np.add.at(cntq, (score_, sslot, squart), 1)
    tcnt_q = np.ceil(cntq / P).astype(np.int64).max(axis=0)  # [NT, NQUART]
    # ensure at least one subtile per slot overall (empty tiles)
    for s in range(NT):
        if tcnt_q[s].sum() == 0:
            tcnt_q[s, 0] = 1

    # processing order: slot-major, quarter runs inside each slot
    sub_tile = []     # slot of each subtile
    sub_quart = []
    for s in range(NT):
        for q in range(NQUART):
            for _ in range(int(tcnt_q[s, q])):
                sub_tile.append(s)
                sub_quart.append(q)
    ET = len(sub_tile)
    sub_tile = np.array(sub_tile)
    sub_quart = np.array(sub_quart)
    # subtile index ranges per slot
    slot_first = np.zeros(NT, np.int64)
    slot_nsub = np.zeros(NT, np.int64)
    for s in range(NT):
        slot_nsub[s] = tcnt_q[s].sum()
    slot_first[1:] = np.cumsum(slot_nsub)[:-1]

    # gather instructions: per (slot, quarter) run, split into <=MAXSUB
    # chunks. Each instr: (quarter, first_subtile, nsub)
    instrs = []
    st = 0
    for s in range(NT):
        for q in range(NQUART):
            k = int(tcnt_q[s, q])
            o = 0
            while o < k:
                take = min(MAXSUB, k - o)
                instrs.append((q, st + o, take))
                o += take
            st += k
    NI_E = len(instrs)

    # fill per-core edge slots + one-hots + int16 indices
    kvidx = np.zeros((NCORES, ET * P), np.int32)  # quarter-relative rows
    ohm = np.zeros((NCORES, ET * P, P), bf)
    ohtm = np.zeros((NCORES, ET, P, P), bf)  # [sub, node, edge]
    # per (core, slot, quarter) edge ranges in the sorted arrays
    gstart = np.zeros((NCORES, NT, NQUART), np.int64)
    np.cumsum(cntq.ravel()[:-1], out=gstart.ravel()[1:])
    # subtile slot offsets: subtile j covers slots [j*P, (j+1)*P)
    sub_of_sq = {}
    st = 0
    for s in range(NT):
        for q in range(NQUART):
            sub_of_sq[(s, q)] = st
            st += int(tcnt_q[s, q])
    for c in range(NCORES):
        for s in range(NT):
            for q in range(NQUART):
                n = int(cntq[c, s, q])
                if n == 0:
                    continue
                e0 = int(gstart[c, s, q])
                o = sub_of_sq[(s, q)] * P
                sl = np.arange(o, o + n)
                kvidx[c, sl] = (srow_m[e0:e0 + n] - q * QR).astype(np.int32)
                cl = (scol[e0:e0 + n] - c * NLOC).astype(np.int32)
                ohm[c, sl, cl % P] = 1
                ohtm[c, sl // P, cl % P, sl % P] = 1

    # int16 idx tensors per gather instruction, padded cols to MAXSUB*8
    SCW = MAXSUB * P // 16
    eidx16 = np.zeros((NCORES, NI_E, P, SCW), np.int16)
    for c in range(NCORES):
        for ii, (q, s0, nsub) in enumerate(instrs):
            vals = kvidx[c, s0 * P:(s0 + nsub) * P]
            eidx16[c, ii, :, :nsub * 8] = _pack_idx16(vals)

    # oh: [sub, slot(part), node(free)] for the agg matmul lhsT;
    # oht: [sub, node(part), slot(free)] for the qsel matmul lhsT.
    oh_hw = np.ascontiguousarray(ohm.reshape(NCORES, ET, P, P))
    oht_hw = np.ascontiguousarray(ohtm)

    # node features (zero-padded, transposed tiles, slot order)
    xs = np.zeros((NCORES, NLOC, IN_C), np.float32)
    rws = np.zeros((NCORES, NLOC, RWD), np.float32)
    for c in range(NCORES):
        lo = c * NLOC
        hi = min(N, lo + NLOC)
        if hi > lo:
            xs[c, :hi - lo] = x[lo:hi]
            rws[c, :hi - lo] = rw[lo:hi]
    ci = np.arange(NCORES)[:, None]
    xT_hw = np.ascontiguousarray(
        xs.reshape(NCORES, NT, P, IN_C)[ci, perm].transpose(
            0, 1, 3, 2)).astype(bf)
    rwT_hw = np.ascontiguousarray(
        rws.reshape(NCORES, NT, P, RWD)[ci, perm].transpose(
            0, 1, 3, 2)).astype(bf)

    # ---- pairs: 16 (src-quarter, dst-quarter) buckets ----
    NPAIR = psrc.shape[0]
    PLOC = math.ceil(NPAIR / NCORES)
    plocs = [max(0, min(PLOC, NPAIR - c * PLOC)) for c in range(NCORES)]
    mpsrc = slot_row(psrc)
    mpdst = slot_row(pdst)
    bidx = (mpsrc // QR) * NQUART + (mpdst // QR)   # [NPAIR] bucket 0..15
    # per-core bucket membership and shared per-bucket subtile counts
    bmax = np.zeros(16, np.int64)
    core_b = []
    for c in range(NCORES):
        lo, hi = c * PLOC, c * PLOC + plocs[c]
        bc = bidx[lo:hi]
        cnts = np.bincount(bc, minlength=16)
        bmax = np.maximum(bmax, np.ceil(cnts / P).astype(np.int64))
        core_b.append((lo, bc))
    bsub = bmax                      # shared subtiles per bucket
    NPS = int(bsub.sum())
    # pair gather instructions per bucket (src side + dst side share
    # slots): per bucket, ceil(nsub/MAXSUB) instrs per side
    pinstrs = []                     # (qsrc, qdst, first_subtile, nsub)
    bfirst = np.zeros(16, np.int64)
    st = 0
    for b in range(16):
        bfirst[b] = st
        o = 0
        while o < int(bsub[b]):
            take = min(MAXSUB, int(bsub[b]) - o)
            pinstrs.append((b // NQUART, b % NQUART, st + o, take))
            o += take
        st += int(bsub[b])
    NI_P = len(pinstrs)

    psidx = np.zeros((NCORES, NPS * P), np.int32)
    pdidx = np.zeros((NCORES, NPS * P), np.int32)
    pperm = np.full((NCORES, NPS * P), -1, np.int64)  # slot -> local pair
    for c in range(NCORES):
        lo, bc = core_b[c]
        for b in range(16):
            sel = np.where(bc == b)[0]          # local pair indices
            o = int(bfirst[b]) * P
            psidx[c, o:o + len(sel)] = (mpsrc[lo + sel]
                                        - (b // NQUART) * QR)
            pdidx[c, o:o + len(sel)] = (mpdst[lo + sel]
                                        - (b % NQUART) * QR)
            pperm[c, o:o + len(sel)] = sel
    pidx16 = np.zeros((NCORES, NI_P, P, 2 * SCW), np.int16)
    for c in range(NCORES):
        for ii, (qs, qd, s0, nsub) in enumerate(pinstrs):
            pidx16[c, ii, :, :nsub * 8] = _pack_idx16(
                psidx[c, s0 * P:(s0 + nsub) * P])
            pidx16[c, ii, :, SCW:SCW + nsub * 8] = _pack_idx16(
                pdidx[c, s0 * P:(s0 + nsub) * P])

    return dict(
        N=N, IN_C=IN_C, RWD=RWD, NT=NT, NLOC=NLOC, NPADT=NPADT, QR=QR,
        ET=ET, NI_E=NI_E, instrs=instrs, SCW=SCW,
        NG=NG, groups=groups, gbase=gbase,
        slot_first=slot_first, slot_nsub=slot_nsub,
        sub_tile=sub_tile, sub_quart=sub_quart, tcnt_q=tcnt_q,
        NPS=NPS, NI_P=NI_P, pinstrs=pinstrs, bsub=bsub, bfirst=bfirst,
        pperm=pperm, PLOC=PLOC, plocs=plocs, NPAIR=NPAIR,
        W1=W1, W2=W2, bcat=bcat, Wcat=Wcat, bqkvs=bqkvs,
        ln_g=ln_g, ln_b=ln_b,
        xT=xT_hw, rwT=rwT_hw, eidx=eidx16, oh=oh_hw, oht=oht_hw,
        pidx=pidx16,
    )
